# revision 37
# baseline (speedup 1.0000x reference)
"""ALiBi attention (B=2, T=2048, D=2048, H=16) on 8 TRN2 NeuronCores.

Sharding: tensor-parallel over heads. Core c owns global heads {c, c+8}
(2 heads/core). Each core:
  1. projects q,k,v for its heads from the full activations (bf16 matmuls,
     f32 accumulate),
  2. runs causal ALiBi attention for its heads (both batches) using a
     bound-shifted softmax (no row-max pass: softmax is invariant to
     per-row constants, so exp(S + slope*j - slope*i - C) with a static
     safe bound replaces max-subtraction),
  3. AllToAll-reshards attention outputs from head-sharded to
     token-sharded,
  4. computes the final Wo projection for its 512-token slice.
Host concatenates the 8 token slices.
"""

import os
import sys

for _p in ("/opt/trn_rl_repo", "/root/.axon_site/_ro/trn_rl_repo"):
    if os.path.isdir(_p) and _p not in sys.path:
        sys.path.insert(0, _p)

import numpy as np
import ml_dtypes

B = 2
T = 2048
D = 2048
H = 16
DH = 128
NCORES = 8
TOKS = B * T  # 4096
KC = 16  # number of 128-row contraction chunks of D
CBOUND = 8.0  # safe upper margin for max_j (q.k/sqrt(dh) + slope*(j-i))

NP_BF16 = ml_dtypes.bfloat16

_CACHE = {}


def _build_nc(reps=1, rep_phases=(1, 2, 3, 4), prelude_phases=(),
              attn_mode="full", trace_sim=False):
    import concourse.bass as bass
    import concourse.tile as tile
    from concourse import bacc, mybir
    from contextlib import ExitStack

    f32 = mybir.dt.float32
    bf16 = mybir.dt.bfloat16
    P = 128

    nc = bacc.Bacc("TRN2", target_bir_lowering=False, debug=False,
                   num_devices=NCORES)

    xT_d = nc.dram_tensor("xT", [D, TOKS], bf16, kind="ExternalInput")
    wqT_d = nc.dram_tensor("wqT", [D, 256], bf16, kind="ExternalInput")
    wkT_d = nc.dram_tensor("wkT", [D, 256], bf16, kind="ExternalInput")
    wvT_d = nc.dram_tensor("wvT", [D, 256], bf16, kind="ExternalInput")
    woT_d = nc.dram_tensor("woT", [D, D], bf16, kind="ExternalInput")
    gg_d = nc.dram_tensor("gdecay", [2, T], bf16, kind="ExternalInput")
    gd_d = nc.dram_tensor("gdiag", [2, 128, 128], bf16,
                          kind="ExternalInput")
    ab_d = nc.dram_tensor("abias", [2, 128], f32, kind="ExternalInput")
    id_d = nc.dram_tensor("ident", [128, 128], bf16, kind="ExternalInput")
    out_d = nc.dram_tensor("out", [TOKS // NCORES, D], f32,
                           kind="ExternalOutput")

    with tile.TileContext(nc, trace_sim=trace_sim) as tc, ExitStack() as top:
        consts = top.enter_context(tc.tile_pool(name="consts", bufs=1))
        weights = top.enter_context(tc.tile_pool(name="weights", bufs=1))
        qkv = top.enter_context(tc.tile_pool(name="qkv", bufs=1))
        dram = top.enter_context(tc.tile_pool(name="dram", bufs=1,
                                              space="DRAM"))

        # ---- constants (tiles only; DMAs deferred past projections) ----
        gg_t = consts.tile([P, 2, T], bf16, name="gg_t")
        gd_t = consts.tile([P, 2, 128], bf16, name="gd_t")
        ab_t = consts.tile([P, 2], f32, name="ab_t")
        id_t = consts.tile([P, 128], bf16, name="id_t")

        # ---- weights ----
        wq_t = weights.tile([P, KC, 256], bf16, name="wq_t")
        wk_t = weights.tile([P, KC, 256], bf16, name="wk_t")
        wv_t = weights.tile([P, KC, 256], bf16, name="wv_t")
        for (w_t, w_d) in ((wq_t, wqT_d), (wk_t, wkT_d), (wv_t, wvT_d)):
            nc.sync.dma_start(
                out=w_t, in_=w_d.ap().rearrange("(kc p) m -> p kc m", p=P))
        wo_t = weights.tile([P, KC, D], bf16, name="wo_t")

        # ---- persistent qkv activations (bf16, feature-major q/k) ----
        qT_t = qkv.tile([P, 2, TOKS], bf16, name="qT_t")
        kT_t = qkv.tile([P, 2, TOKS], bf16, name="kT_t")
        v_t = qkv.tile([P, TOKS // P, 256], bf16, name="v_t")

        # ---- A2A bounce buffers ----
        bounce_in = dram.tile([TOKS, 256], bf16, name="bounce_in")
        bounce_out = dram.tile([TOKS, 256], bf16, name="bounce_out")

        xT_r = xT_d.ap().rearrange("(kc p) t -> p kc t", p=P)

        if prelude_phases:
            _env = dict(locals())
            _env["_rep"] = -1
            _env["attn_mode"] = "full"
            _emit_body(nc, tc, bass, mybir, ExitStack, _env, prelude_phases)
        for _rep in range(reps):
            _env = dict(locals())
            _env["_rep"] = _rep
            _emit_body(nc, tc, bass, mybir, ExitStack, _env, rep_phases)

    nc.compile()
    return nc


def _emit_body(nc, tc, bass, mybir, ExitStack, env, phases):
    f32 = mybir.dt.float32
    bf16 = mybir.dt.bfloat16
    P = 128
    wq_t, wk_t, wv_t, wo_t = env["wq_t"], env["wk_t"], env["wv_t"], env["wo_t"]
    qT_t, kT_t, v_t = env["qT_t"], env["kT_t"], env["v_t"]
    gg_t, gd_t, id_t = env["gg_t"], env["gd_t"], env["id_t"]
    ab_t = env["ab_t"]
    bounce_in, bounce_out = env["bounce_in"], env["bounce_out"]
    xT_r, woT_d, out_d = env["xT_r"], env["woT_d"], env["out_d"]
    _rep = env["_rep"]
    attn_mode = env["attn_mode"]
    do_sm = attn_mode in ("full", "softmax_only")
    do_pv = attn_mode in ("full", "pe_only")

    if 1 in phases:
        # ================= phase 1: q/k/v projections =================
        with ExitStack() as proj:
            xpool = proj.enter_context(tc.tile_pool(name="xpool", bufs=2))
            pqk = proj.enter_context(
                tc.tile_pool(name="pqk", bufs=4, space="PSUM"))
            pv = proj.enter_context(
                tc.tile_pool(name="pv", bufs=2, space="PSUM"))
            for tch in range(TOKS // 512):
                sl = slice(tch * 512, (tch + 1) * 512)
                xt = xpool.tile([P, KC, 512], bf16, name="xt")
                nc.sync.dma_start(out=xt, in_=xT_r[:, :, sl])
                # q/k: stationary = weight chunk, moving = xT -> [dh, tok]
                for (w_t, dst) in ((wq_t, qT_t), (wk_t, kT_t)):
                    for s in range(2):
                        ps = pqk.tile([P, 512], f32, name="ps", tag="pqk")
                        for kc in range(KC):
                            nc.tensor.matmul(
                                ps, w_t[:, kc, s * 128:(s + 1) * 128],
                                xt[:, kc, :],
                                start=(kc == 0), stop=(kc == KC - 1))
                        nc.any.tensor_copy(dst[:, s, sl], ps)
                # v: stationary = xT chunk, moving = wvT -> [tok, dv]
                for sub in range(4):
                    psv = pv.tile([P, 256], f32, name="psv", tag="pv")
                    for kc in range(KC):
                        nc.tensor.matmul(
                            psv, xt[:, kc, sub * 128:(sub + 1) * 128],
                            wv_t[:, kc, :],
                            start=(kc == 0), stop=(kc == KC - 1))
                    nc.any.tensor_copy(v_t[:, tch * 4 + sub, :], psv)

    # attention constants + wo loaded after projections so they don't
    # delay the first xT tiles
    if not getattr(nc, "_consts_emitted", False):
        nc._consts_emitted = True
        gg_d, gd_d, ab_d, id_d = (env["gg_d"], env["gd_d"], env["ab_d"],
                                  env["id_d"])
        for s in range(2):
            row = gg_d.ap()[s]
            bcast = bass.AP(tensor=row.tensor, offset=row.offset,
                            ap=[[0, P]] + list(row.ap))
            nc.sync.dma_start(out=gg_t[:, s, :], in_=bcast)
            nc.sync.dma_start(out=gd_t[:, s, :], in_=gd_d.ap()[s])
            nc.sync.dma_start(out=ab_t[:, s:s + 1], in_=ab_d.ap()[s])
        nc.sync.dma_start(out=id_t, in_=id_d.ap())
        nc.sync.dma_start(
            out=wo_t, in_=woT_d.ap().rearrange("(kc p) m -> p kc m", p=P))

    if 2 in phases:
        # ================= phase 2: attention =================
        with ExitStack() as att:
            prow = att.enter_context(tc.tile_pool(name="prow", bufs=2))
            lpool = att.enter_context(tc.tile_pool(name="lpool", bufs=6))
            osb = att.enter_context(tc.tile_pool(name="osb", bufs=3))
            ptsb = att.enter_context(tc.tile_pool(name="ptsb", bufs=3))
            psS = att.enter_context(
                tc.tile_pool(name="psS", bufs=4, space="PSUM"))
            psT = att.enter_context(
                tc.tile_pool(name="psT", bufs=2, space="PSUM"))
            psO = att.enter_context(
                tc.tile_pool(name="psO", bufs=2, space="PSUM"))

            for s in range(2):
                for b in range(2):
                    boff = b * T
                    for it in range(T // 128):
                        jext = (it + 1) * 128
                        ngrp = (jext + 511) // 512
                        if do_sm:
                            pr = prow.tile([P, T], bf16, name="pr")
                        qsl = qT_t[:, s, boff + it * 128: boff + (it + 1) * 128]
                        for g in range(ngrp):
                            w = min(512, jext - g * 512)
                            ps = psS.tile([P, 512], f32, name="ps", tag="s")
                            nc.tensor.matmul(
                                ps[:, :w], qsl,
                                kT_t[:, s, boff + g * 512: boff + g * 512 + w],
                                start=True, stop=True)
                            if not do_sm:
                                continue
                            # P1 = exp(S - slope*(di-64)); the per-row half
                            # of the ALiBi bias rides the ACT bias port.
                            nc.scalar.activation(
                                out=pr[:, g * 512: g * 512 + w],
                                in_=ps[:, :w],
                                func=mybir.ActivationFunctionType.Exp,
                                bias=ab_t[:, s:s + 1])
                        # P2 = P1 * alibi-decay (mask folded into gdiag),
                        # row-sums l via accum.
                        if do_sm:
                            li = lpool.tile([P, 1], f32, name="li", tag="li")
                            lt = lpool.tile([P, 1], f32, name="lt", tag="l")
                            if it > 0:
                                lm = lpool.tile([P, 1], f32, name="lm",
                                                tag="lm")
                                nc.vector.scalar_tensor_tensor(
                                    out=pr[:, :jext - 128],
                                    in0=pr[:, :jext - 128],
                                    scalar=1.0,
                                    in1=gg_t[:, s, T - jext: T - 128],
                                    op0=mybir.AluOpType.mult,
                                    op1=mybir.AluOpType.mult,
                                    accum_out=lm)
                            ld = lpool.tile([P, 1], f32, name="ld", tag="ld")
                            nc.vector.scalar_tensor_tensor(
                                out=pr[:, jext - 128:jext],
                                in0=pr[:, jext - 128:jext],
                                scalar=1.0,
                                in1=gd_t[:, s, :],
                                op0=mybir.AluOpType.mult,
                                op1=mybir.AluOpType.mult,
                                accum_out=ld)
                            if it > 0:
                                nc.vector.tensor_add(lt, lm, ld)
                            else:
                                lt = ld
                            nc.vector.reciprocal(li, lt)
                        if not do_pv:
                            continue
                        po = psO.tile([P, 128], f32, name="po", tag="o")
                        for g in range(ngrp):
                            w = min(512, jext - g * 512)
                            nsub = w // 128
                            pt = psT.tile([P, 512], bf16, name="pt", tag="t")
                            for u in range(nsub):
                                tsrc = (pr[:, g * 512 + u * 128:
                                           g * 512 + (u + 1) * 128]
                                        if do_sm else
                                        kT_t[:, s, g * 512 + u * 128:
                                             g * 512 + (u + 1) * 128])
                                nc.tensor.transpose(
                                    pt[:, u * 128:(u + 1) * 128], tsrc, id_t)
                            pts = ptsb.tile([P, 512], bf16, name="pts")
                            i32 = mybir.dt.int32
                            nc.vector.tensor_copy(
                                pts[:, :w].bitcast(i32), pt[:, :w].bitcast(i32))
                            for u in range(nsub):
                                jc = g * 4 + u
                                nc.tensor.matmul(
                                    po, pts[:, u * 128:(u + 1) * 128],
                                    v_t[:, b * 16 + jc,
                                        s * 128:(s + 1) * 128],
                                    start=(jc == 0), stop=(jc == it))
                        ot = osb.tile([P, 128], bf16, name="ot")
                        if do_sm:
                            nc.vector.tensor_scalar_mul(ot, po, li)
                        else:
                            nc.vector.tensor_copy(ot, po)
                        nc.sync.dma_start(
                            out=bounce_in[boff + it * 128:
                                          boff + (it + 1) * 128,
                                          s * 128:(s + 1) * 128],
                            in_=ot)

    if 3 in phases:
        # ================= phase 3: AllToAll reshard =================
        nc.gpsimd.collective_compute(
            "AllToAll", mybir.AluOpType.bypass,
            replica_groups=[list(range(NCORES))],
            ins=[bounce_in.opt()], outs=[bounce_out.opt()])

    if 4 in phases:
        # ================= phase 4: output projection =================
        with ExitStack() as p4:
            afp = p4.enter_context(tc.tile_pool(name="afp", bufs=18))
            aft = p4.enter_context(tc.tile_pool(name="aft", bufs=18))
            p4s = p4.enter_context(tc.tile_pool(name="p4s", bufs=3))
            p4t = p4.enter_context(
                tc.tile_pool(name="p4t", bufs=2, space="PSUM"))
            p4o = p4.enter_context(
                tc.tile_pool(name="p4o", bufs=2, space="PSUM"))
            for tb in range(4):
                afts = []
                for bi in range(KC):
                    r, s = bi // 2, bi % 2
                    af = afp.tile([P, 128], bf16, name="af", tag="af")
                    nc.sync.dma_start(
                        out=af,
                        in_=bounce_out[r * 512 + tb * 128:
                                       r * 512 + (tb + 1) * 128,
                                       s * 128:(s + 1) * 128])
                    tp = p4t.tile([P, 128], bf16, name="tp", tag="tp")
                    nc.tensor.transpose(tp, af, id_t)
                    afx = aft.tile([P, 128], bf16, name="afx", tag="afx")
                    nc.any.tensor_copy(afx, tp)
                    afts.append(afx)
                for oc in range(4):
                    pf = p4o.tile([P, 512], f32, name="pf", tag="pf")
                    for bi in range(KC):
                        nc.tensor.matmul(
                            pf, afts[bi],
                            wo_t[:, bi, oc * 512:(oc + 1) * 512],
                            start=(bi == 0), stop=(bi == KC - 1))
                    ofs = p4s.tile([P, 512], f32, name="ofs", tag="ofs")
                    nc.any.tensor_copy(ofs, pf)
                    nc.sync.dma_start(
                        out=out_d.ap()[tb * 128:(tb + 1) * 128,
                                       oc * 512:(oc + 1) * 512],
                        in_=ofs)


def _get_nc():
    if "nc" not in _CACHE:
        import concourse.mybir as mybir  # noqa: F401
        _CACHE["nc"] = _build_nc()
    return _CACHE["nc"]


def _make_in_maps(x, Wq, Wk, Wv, Wo):
    x = np.asarray(x, np.float32)
    Wq = np.asarray(Wq, np.float32)
    Wk = np.asarray(Wk, np.float32)
    Wv = np.asarray(Wv, np.float32)
    Wo = np.asarray(Wo, np.float32)

    xT = np.ascontiguousarray(x.reshape(TOKS, D).T).astype(NP_BF16)
    slopes = (0.5 ** (np.arange(1, H + 1) * 8.0 / H)).astype(np.float32)
    jj = np.arange(T, dtype=np.float32)
    dj = np.arange(128, dtype=np.float32)
    perm = [r + 8 * s for r in range(8) for s in range(2)]
    woT = np.ascontiguousarray(Wo.T).reshape(H, DH, D)[perm] \
        .reshape(D, D).astype(NP_BF16)
    ident = np.eye(128, dtype=NP_BF16)
    scale = np.float32(1.0 / np.sqrt(DH))
    causal = (dj[None, :] <= dj[:, None])

    in_maps = []
    for c in range(NCORES):
        heads = [c, c + 8]
        wqT = np.concatenate(
            [Wq[h * DH:(h + 1) * DH].T for h in heads], 1) * scale
        wkT = np.concatenate([Wk[h * DH:(h + 1) * DH].T for h in heads], 1)
        wvT = np.concatenate([Wv[h * DH:(h + 1) * DH].T for h in heads], 1)
        # With the per-row exp bias -slope*(di-64), the decay tables are
        # shifted +64 so that P1*G == exp(S + slope*(j-i)) exactly.
        # gdiag additionally folds in the causal mask.
        gg = np.stack([np.exp(slopes[h] * (jj - T + 64)) for h in heads])
        gd = np.stack(
            [np.where(causal, np.exp(slopes[h] * (dj[None, :] - 64)), 0.0)
             for h in heads])
        ab = np.stack([-slopes[h] * (dj - 64) for h in heads]) \
            .astype(np.float32)
        in_maps.append({
            "xT": xT,
            "wqT": np.ascontiguousarray(wqT).astype(NP_BF16),
            "wkT": np.ascontiguousarray(wkT).astype(NP_BF16),
            "wvT": np.ascontiguousarray(wvT).astype(NP_BF16),
            "woT": woT,
            "gdecay": np.ascontiguousarray(gg).astype(NP_BF16),
            "gdiag": np.ascontiguousarray(gd).astype(NP_BF16),
            "abias": np.ascontiguousarray(ab),
            "ident": ident,
        })
    return in_maps


LAST_RESULTS = None


def kernel(x, Wq, Wk, Wv, Wo):
    global LAST_RESULTS
    from concourse import bass_utils

    nc = _get_nc()
    in_maps = _make_in_maps(x, Wq, Wk, Wv, Wo)
    res = bass_utils.run_bass_kernel_spmd(
        nc, in_maps, core_ids=list(range(NCORES)))
    LAST_RESULTS = res
    out = np.concatenate(
        [np.asarray(res.results[c]["out"], np.float32)
         for c in range(NCORES)], 0)
    return out.reshape(B, T, D)
